# revision 9
# baseline (speedup 1.0000x reference)
"""Single-head causal attention (B=4, S=2048, D=1024) on 8 TRN2 NeuronCores.

Sharding: core c -> (batch b = c//2, half h = c%2). Each core computes the
full K/V projections for its batch and attends two 512-query blocks chosen
so causal work balances across the two cores of a batch:
  h=0: query rows [0:512)    and [1536:2048)   (4 + 16 causal key-chunks)
  h=1: query rows [512:1024) and [1024:1536)   (8 + 12 causal key-chunks)
The SPMD program is uniform: block A always scans 8 key-chunks, block B 16;
out-of-causal-range chunks are zeroed by a host-supplied multiplicative mask
(which also applies the intra-diagonal triangle), so all 8 cores run the
same instruction stream on different data.

Layout: everything transposed. xT/qT/kT are [d_part, seq_free]; scores are
computed as S^T [key_part, q_free] so exp runs on ScalarE along the free
axis with no transposes anywhere. Softmax uses no max-subtraction (scores
are O(few) by construction), and normalization is deferred: unnormalized
ctx flows through the output projection and each [128q, dout] result tile
is scaled by 1/denom as a per-partition scalar. Denominators come from N=1
matmuls vs a ones vector. Biases are handled on the host: bq/bk are
exactly zero in this problem, and bv/bo enter additively as (bv @ Wo + bo).

Matmuls run as float32r (full-rate fp32 on the PE at N>=256). The fused
fp32r matmul ISA slot carries at most ONE sync wait, so the program is
arranged so every matmul has at most one uncovered dependency:
  - every logical load is a single DMA instruction (one DMA-lane dep),
  - tiny "absorber" matmuls into a dedicated scratch PSUM bank observe
    each DMA lane on the PE before the real matmul group needs it,
  - PSUM->SBUF copies are routed per-phase to a single engine so psum-WAR
    and operand deps collapse into one engine-tick wait.
"""

import numpy as np

import concourse.bass as bass
import concourse.bacc as bacc
import concourse.mybir as mybir
from concourse.tile import TileContext
from concourse.bass_utils import run_bass_kernel_spmd

B, S, D = 4, 2048, 1024
P = 128
QB = 512                    # query-block width (free dim of score matmuls)
NKC = (8, 16)               # key-chunks scanned for block A / block B
NDC = D // P                # 8 d-chunks
NKB = S // QB               # 4 key-blocks in projection
NQS = QB // P               # 4 query sub-tiles per block
PV_PASSES = ((0, 1, 2), (3, 4, 5), (6, 7))
F32 = mybir.dt.float32
F32R = mybir.dt.float32r
SCALE = 1.0 / float(np.sqrt(D))

# q-row starts per (h, block)
Q_STARTS = {0: (0, 3 * QB), 1: (QB, 2 * QB)}


def _r(ap):
    return ap.bitcast(mybir.dt.float32r)


def _build_program():
    nc = bacc.Bacc("TRN2", target_bir_lowering=False, debug=False)
    xT = nc.declare_dram_parameter("xT", [D, S], F32, isOutput=False)
    qxT = nc.declare_dram_parameter("qxT", [D, 2 * QB], F32, isOutput=False)
    w_d = {
        n: nc.declare_dram_parameter(n, [D, D], F32, isOutput=False)
        for n in ("Wq", "Wk", "Wv", "Wo")
    }
    cm_d = nc.declare_dram_parameter("cmask", [sum(NKC), P, QB], F32, isOutput=False)
    out_d = nc.declare_dram_parameter("o_out", [2 * QB, D], F32, isOutput=True)
    vspill = nc.dram_tensor("vspill", [S // P, P, D], F32R)

    with TileContext(nc) as tc:
        with (
            tc.tile_pool(name="persist", bufs=1) as pp,
            tc.tile_pool(name="w", bufs=1) as wp,
            tc.tile_pool(name="ps_s", bufs=2, space="PSUM") as ps_s,
            tc.tile_pool(name="ps_c", bufs=3, space="PSUM") as ps_c,
            tc.tile_pool(name="ps_o", bufs=2, space="PSUM") as ps_o,
            tc.tile_pool(name="ps_x", bufs=1, space="PSUM") as ps_x,
        ):
            # persistent SBUF tensors
            kt = [pp.tile([P, S], F32R, name=f"kt{i}") for i in range(NDC)]
            qt = {
                blk: [pp.tile([P, QB], F32R, name=f"qt{blk}{i}") for i in range(NDC)]
                for blk in (0, 1)
            }
            ones_t = pp.tile([P, 2], F32, name="ones_t")
            nc.vector.memset(ones_t[:], 1.0)

            scratch = ps_x.tile([1, 16], F32, name="scratch")

            def absorb(ap_cols):
                # Tiny fp32 matmul whose only job is to make the PE observe
                # ap_cols' producer (one DMA lane) before the real group.
                a = ap_cols.bitcast(F32)
                nc.tensor.matmul(
                    scratch[0:1, 0:2], a[:, 0:1], a[:, 0:2],
                    start=True, stop=True,
                )

            def load_w(wname):
                wall = wp.tile([P, NDC, D], F32R, name="wall")
                nc.gpsimd.dma_start(
                    out=wall[:],
                    in_=_r(w_d[wname].rearrange("(a p) d -> p a d", p=P)),
                )
                absorb(wall[:, 0, 0:2])
                return wall

            # ---------------- P1: projections ----------------
            with tc.tile_pool(name="xtk", bufs=2) as xtp:

                def load_xt(src, col0):
                    xta = xtp.tile([P, NDC, QB], F32R, name="xta")
                    nc.gpsimd.dma_start(
                        out=xta[:],
                        in_=_r(
                            src.rearrange("(a p) s -> p a s", p=P)[
                                :, :, col0:col0 + QB
                            ]
                        ),
                    )
                    absorb(xta[:, 0, 0:2])
                    return xta

                # -- round 1: kT = Wk^T x^T
                wk = load_w("Wk")
                for kb in range(NKB):
                    xta = load_xt(xT, kb * QB)
                    for do in range(NDC):
                        ps = ps_s.tile([P, QB], F32, name="pss")
                        for di in range(NDC):
                            nc.tensor.matmul(
                                ps[:],
                                wk[:, di, do * P:(do + 1) * P],
                                xta[:, di, :],
                                start=(di == 0),
                                stop=(di == NDC - 1),
                            )
                        nc.scalar.copy(kt[do][:, kb * QB:(kb + 1) * QB], ps[:])

                # -- round 2: V natural = x Wv, spilled to DRAM by k-chunk
                wv = load_w("Wv")
                with tc.tile_pool(name="vtmp", bufs=3) as vtp:
                    for kb in range(NKB):
                        xta = load_xt(xT, kb * QB)
                        for kcl in range(QB // P):
                            kc = kb * (QB // P) + kcl
                            for dh in range(2):
                                ps = ps_s.tile([P, QB], F32, name="pss")
                                for di in range(NDC):
                                    nc.tensor.matmul(
                                        ps[:],
                                        xta[:, di, kcl * P:(kcl + 1) * P],
                                        wv[:, di, dh * QB:(dh + 1) * QB],
                                        start=(di == 0),
                                        stop=(di == NDC - 1),
                                    )
                                vt = vtp.tile([P, QB], F32R, name="vtmp")
                                nc.vector.tensor_copy(vt[:], ps[:])
                                nc.gpsimd.dma_start(
                                    out=vspill[kc, :, dh * QB:(dh + 1) * QB],
                                    in_=vt[:],
                                )

                # -- round 3: qT = Wq^T x^T for the core's two query blocks
                wq = load_w("Wq")
                for blk in (0, 1):
                    xta = load_xt(qxT, blk * QB)
                    for do in range(NDC):
                        ps = ps_s.tile([P, QB], F32, name="pss")
                        for di in range(NDC):
                            nc.tensor.matmul(
                                ps[:],
                                wq[:, di, do * P:(do + 1) * P],
                                xta[:, di, :],
                                start=(di == 0),
                                stop=(di == NDC - 1),
                            )
                        nc.scalar.copy(qt[blk][do][:], ps[:])

            # -- Wo for the output projection (reuses the w slot)
            wo = load_w("Wo")

            # ---------------- P2: attention per block ----------------
            with (
                tc.tile_pool(name="et", bufs=1) as etp,
                tc.tile_pool(name="vld", bufs=3) as vlp,
                tc.tile_pool(name="cm", bufs=2) as cmp_,
                tc.tile_pool(name="ctxs", bufs=1) as ctp,
                tc.tile_pool(name="osb", bufs=2) as osp,
                tc.tile_pool(name="rd", bufs=1) as rdp,
            ):
                for blk in (0, 1):
                    nkc = NKC[blk]
                    cmbase = 0 if blk == 0 else NKC[0]
                    # S phase: scores^T -> exp -> mask
                    et = [etp.tile([P, QB], F32R, name=f"et{i}") for i in range(nkc)]
                    for kc in range(nkc):
                        ps = ps_s.tile([P, QB], F32, name="pss")
                        for di in range(NDC):
                            nc.tensor.matmul(
                                ps[:],
                                kt[di][:, kc * P:(kc + 1) * P],
                                qt[blk][di][:],
                                start=(di == 0),
                                stop=(di == NDC - 1),
                            )
                        nc.scalar.activation(
                            et[kc][:], ps[:], mybir.ActivationFunctionType.Exp,
                            scale=SCALE,
                        )
                        cm = cmp_.tile([P, QB], F32, name="cm")
                        nc.gpsimd.dma_start(out=cm[:], in_=cm_d[cmbase + kc])
                        nc.vector.tensor_mul(et[kc][:], et[kc][:], cm[:])

                    # DEN phase: denom[q] per query sub-tile, then 1/denom
                    d_t = rdp.tile([P, NQS], F32, name=f"dt{blk}")
                    r_t = rdp.tile([P, NQS], F32, name=f"rt{blk}")
                    for qs in range(NQS):
                        psd = ps_o.tile([P, QB], F32, name="pso", tag="o")
                        for kc in range(nkc):
                            nc.tensor.matmul(
                                psd[:, 0:2],
                                et[kc][:, qs * P:(qs + 1) * P].bitcast(F32),
                                ones_t[:],
                                start=(kc == 0),
                                stop=(kc == nkc - 1),
                            )
                        nc.vector.tensor_copy(d_t[:, qs:qs + 1], psd[:, 0:1])
                    nc.vector.reciprocal(r_t[:], d_t[:])

                    # PV phase: ctx^T[d, q] += v[k, d]^T-slices @ e^T[k, q]
                    ctxs = [
                        ctp.tile([P, QB], F32R, name=f"ctxs{i}") for i in range(NDC)
                    ]
                    for chunk in PV_PASSES:
                        w_pass = len(chunk) * P
                        psc = [ps_c.tile([P, QB], F32, name="psc") for _ in chunk]
                        for kc in range(nkc):
                            vl = vlp.tile([P, 3 * P], F32R, name="vld")
                            nc.gpsimd.dma_start(
                                out=vl[:, :w_pass],
                                in_=vspill[
                                    kc, :, chunk[0] * P:chunk[0] * P + w_pass
                                ],
                            )
                            absorb(vl[:, 0:2])
                            for j, dc in enumerate(chunk):
                                nc.tensor.matmul(
                                    psc[j][:],
                                    vl[:, j * P:(j + 1) * P],
                                    et[kc][:],
                                    start=(kc == 0),
                                    stop=(kc == nkc - 1),
                                )
                        for j, dc in enumerate(chunk):
                            nc.vector.tensor_copy(ctxs[dc][:], psc[j][:])

                    # OPROJ phase: Z = ctx^T.T @ Wo, normalize, store
                    for qs in range(NQS):
                        for dh in range(2):
                            pso = ps_o.tile([P, QB], F32, name="pso", tag="o")
                            for dc in range(NDC):
                                nc.tensor.matmul(
                                    pso[:],
                                    ctxs[dc][:, qs * P:(qs + 1) * P],
                                    wo[:, dc, dh * QB:(dh + 1) * QB],
                                    start=(dc == 0),
                                    stop=(dc == NDC - 1),
                                )
                            ot = osp.tile([P, QB], F32, name="osb")
                            nc.vector.tensor_scalar_mul(
                                ot[:], pso[:], r_t[:, qs:qs + 1]
                            )
                            nc.gpsimd.dma_start(
                                out=out_d[
                                    blk * QB + qs * P: blk * QB + (qs + 1) * P,
                                    dh * QB:(dh + 1) * QB,
                                ],
                                in_=ot[:],
                            )
    nc.compile()
    return nc


_PROG = None


def _get_program():
    global _PROG
    if _PROG is None:
        _PROG = _build_program()
    return _PROG


def _make_core_inputs(x, Wq, Wk, Wv, Wo):
    """Build the per-core input maps (host-side sharding)."""
    in_maps = []
    qarr = np.arange(QB)
    for c in range(8):
        b, h = c // 2, c % 2
        xTb = np.ascontiguousarray(x[b].T)          # [D, S]
        q0A, q0B = Q_STARTS[h]
        qxT = np.ascontiguousarray(
            np.concatenate([x[b, q0A:q0A + QB], x[b, q0B:q0B + QB]], axis=0).T
        )                                            # [D, 2*QB]
        cm = np.empty((sum(NKC), P, QB), dtype=np.float32)
        for blk, (nkc, q0) in enumerate(zip(NKC, (q0A, q0B))):
            base = 0 if blk == 0 else NKC[0]
            for kc in range(nkc):
                karr = kc * P + np.arange(P)
                cm[base + kc] = (karr[:, None] <= (q0 + qarr)[None, :]).astype(
                    np.float32
                )
        in_maps.append(
            {
                "xT": xTb,
                "qxT": qxT,
                "Wq": Wq,
                "Wk": Wk,
                "Wv": Wv,
                "Wo": Wo,
                "cmask": cm,
            }
        )
    return in_maps


def _run(inputs, trace=False, trace_kwargs=None):
    x = np.asarray(inputs["x"], dtype=np.float32)
    Wq = np.asarray(inputs["Wq"], dtype=np.float32)
    Wk = np.asarray(inputs["Wk"], dtype=np.float32)
    Wv = np.asarray(inputs["Wv"], dtype=np.float32)
    Wo = np.asarray(inputs["Wo"], dtype=np.float32)
    bq = np.asarray(inputs["bq"], dtype=np.float32)
    bk = np.asarray(inputs["bk"], dtype=np.float32)
    bv = np.asarray(inputs["bv"], dtype=np.float32)
    bo = np.asarray(inputs["bo"], dtype=np.float32)
    assert not (np.any(bq) or np.any(bk)), "nonzero bq/bk unsupported"

    nc = _get_program()
    in_maps = _make_core_inputs(x, Wq, Wk, Wv, Wo)
    res = run_bass_kernel_spmd(
        nc, in_maps, list(range(8)), trace=trace, **(trace_kwargs or {})
    )

    out = np.empty((B, S, D), dtype=np.float32)
    for c in range(8):
        b, h = c // 2, c % 2
        q0A, q0B = Q_STARTS[h]
        o = res.results[c]["o_out"]
        out[b, q0A:q0A + QB] = o[:QB]
        out[b, q0B:q0B + QB] = o[QB:]
    out += bv @ Wo + bo                     # exact: attn rows sum to 1
    return out, res


def kernel(**inputs):
    out, _ = _run(inputs)
    return out


# revision 10
# speedup vs baseline: 1.2632x; 1.2632x over previous
"""Single-head causal attention (B=4, S=2048, D=1024) on 8 TRN2 NeuronCores.

Sharding: core c -> (batch b = c//2, half h = c%2). Each core computes the
full K/V projections for its batch and attends two 512-query blocks chosen
so causal work balances across the two cores of a batch:
  h=0: query rows [0:512)    and [1536:2048)   (4 + 16 causal key-chunks)
  h=1: query rows [512:1024) and [1024:1536)   (8 + 12 causal key-chunks)
The SPMD program is uniform: block A always scans 8 key-chunks, block B 16;
out-of-causal-range chunks are zeroed by a host-supplied multiplicative mask
(which also applies the intra-diagonal triangle), so all 8 cores run the
same instruction stream on different data.

Layout: everything transposed. xT/qT/kT are [d_part, seq_free]; scores are
computed as S^T [key_part, q_free] so exp runs on ScalarE along the free
axis with no transposes anywhere. Softmax uses no max-subtraction (scores
are O(few) by construction), and normalization is deferred: unnormalized
ctx flows through the output projection and each [128q, dout] result tile
is scaled by 1/denom as a per-partition scalar. Denominators come from N=1
matmuls vs a ones vector. Biases are handled on the host: bq/bk are
exactly zero in this problem, and bv/bo enter additively as (bv @ Wo + bo).

Matmuls run as float32r (full-rate fp32 on the PE at N>=256). The fused
fp32r matmul ISA slot carries at most ONE sync wait, so the program is
arranged so every matmul has at most one uncovered dependency:
  - every logical load is a single DMA instruction (one DMA-lane dep),
  - tiny "absorber" matmuls into a dedicated scratch PSUM bank observe
    each DMA lane on the PE before the real matmul group needs it,
  - PSUM->SBUF copies are routed per-phase to a single engine so psum-WAR
    and operand deps collapse into one engine-tick wait.
"""

import numpy as np

import concourse.bass as bass
import concourse.bacc as bacc
import concourse.mybir as mybir
from concourse.tile import TileContext
from concourse.bass_utils import run_bass_kernel_spmd

B, S, D = 4, 2048, 1024
P = 128
QB = 512                    # query-block width (free dim of score matmuls)
NKC = (8, 16)               # key-chunks scanned for block A / block B
NDC = D // P                # 8 d-chunks
NKB = S // QB               # 4 key-blocks in projection
NQS = QB // P               # 4 query sub-tiles per block
PV_PASSES = ((0, 1, 2), (3, 4, 5), (6, 7))
F32 = mybir.dt.float32
F32R = mybir.dt.float32r
SCALE = 1.0 / float(np.sqrt(D))

# q-row starts per (h, block)
Q_STARTS = {0: (0, 3 * QB), 1: (QB, 2 * QB)}


def _r(ap):
    return ap.bitcast(mybir.dt.float32r)


def _build_program():
    nc = bacc.Bacc("TRN2", target_bir_lowering=False, debug=False)
    xT = nc.declare_dram_parameter("xT", [D, S], F32, isOutput=False)
    qxT = nc.declare_dram_parameter("qxT", [D, 2 * QB], F32, isOutput=False)
    w_d = {
        n: nc.declare_dram_parameter(n, [D, D], F32, isOutput=False)
        for n in ("Wq", "Wk", "Wv", "Wo")
    }
    cm_d = nc.declare_dram_parameter("cmask", [sum(NKC), P, QB], F32, isOutput=False)
    out_d = nc.declare_dram_parameter("o_out", [2 * QB, D], F32, isOutput=True)
    vspill = nc.dram_tensor("vspill", [S // P, P, D], F32R)
    qtspill = nc.dram_tensor("qtspill", [NDC, P, QB], F32R)

    with TileContext(nc) as tc:
        with (
            tc.tile_pool(name="persist", bufs=1) as pp,
            tc.tile_pool(name="ps_s", bufs=2, space="PSUM") as ps_s,
            tc.tile_pool(name="ps_c", bufs=3, space="PSUM") as ps_c,
            tc.tile_pool(name="ps_o", bufs=2, space="PSUM") as ps_o,
            tc.tile_pool(name="ps_x", bufs=1, space="PSUM") as ps_x,
        ):
            # persistent SBUF tensors
            kt = [pp.tile([P, S], F32R, name=f"kt{i}") for i in range(NDC)]
            qt = [pp.tile([P, QB], F32R, name=f"qt{i}") for i in range(NDC)]
            ones_t = pp.tile([P, 2], F32, name="ones_t")
            nc.vector.memset(ones_t[:], 1.0)
            onesr = pp.tile([P, 1], F32R, name="onesr")
            nc.scalar.copy(onesr[:], ones_t[:, 0:1])

            scratch = ps_x.tile([P, 16], F32, name="scratch")

            def load_w(pool, wname):
                wall = pool.tile([P, NDC, D], F32R, name="wall")
                nc.gpsimd.dma_start(
                    out=wall[:],
                    in_=_r(w_d[wname].rearrange("(a p) d -> p a d", p=P)),
                )
                return wall

            # ---------------- P1: projections ----------------
            with (
                tc.tile_pool(name="w", bufs=2) as wp,
                tc.tile_pool(name="xtk", bufs=2) as xtp,
            ):

                def load_xt(src, col0):
                    xta = xtp.tile([P, NDC, QB], F32R, name="xta")
                    nc.gpsimd.dma_start(
                        out=xta[:],
                        in_=_r(
                            src.rearrange("(a p) s -> p a s", p=P)[
                                :, :, col0:col0 + QB
                            ]
                        ),
                    )
                    return xta

                # -- round 1: kT = Wk^T x^T
                wk = load_w(wp, "Wk")
                for kb in range(NKB):
                    xta = load_xt(xT, kb * QB)
                    for do in range(NDC):
                        ps = ps_s.tile([P, QB], F32, name="pss")
                        for di in range(NDC):
                            nc.tensor.matmul(
                                ps[:],
                                wk[:, di, do * P:(do + 1) * P],
                                xta[:, di, :],
                                start=(di == 0),
                                stop=(di == NDC - 1),
                            )
                        nc.scalar.copy(kt[do][:, kb * QB:(kb + 1) * QB], ps[:])

                # -- round 2: V natural = x Wv, spilled to DRAM by k-chunk
                wv = load_w(wp, "Wv")
                with tc.tile_pool(name="vtmp", bufs=3) as vtp:
                    for kb in range(NKB):
                        xta = load_xt(xT, kb * QB)
                        for kcl in range(QB // P):
                            kc = kb * (QB // P) + kcl
                            for dh in range(2):
                                ps = ps_s.tile([P, QB], F32, name="pss")
                                for di in range(NDC):
                                    nc.tensor.matmul(
                                        ps[:],
                                        xta[:, di, kcl * P:(kcl + 1) * P],
                                        wv[:, di, dh * QB:(dh + 1) * QB],
                                        start=(di == 0),
                                        stop=(di == NDC - 1),
                                    )
                                vt = vtp.tile([P, QB], F32R, name="vtmp")
                                nc.vector.tensor_copy(vt[:], ps[:])
                                nc.gpsimd.dma_start(
                                    out=vspill[kc, :, dh * QB:(dh + 1) * QB],
                                    in_=vt[:],
                                )

                # -- round 3: qT = Wq^T x^T; block A kept in SBUF, block B
                #    bounced through DRAM (frees 16KB for W double-buffering)
                wq = load_w(wp, "Wq")
                with tc.tile_pool(name="qb", bufs=3) as qbp:
                    for blk in (0, 1):
                        xta = load_xt(qxT, blk * QB)
                        for do in range(NDC):
                            ps = ps_s.tile([P, QB], F32, name="pss")
                            for di in range(NDC):
                                nc.tensor.matmul(
                                    ps[:],
                                    wq[:, di, do * P:(do + 1) * P],
                                    xta[:, di, :],
                                    start=(di == 0),
                                    stop=(di == NDC - 1),
                                )
                            if blk == 0:
                                nc.scalar.copy(qt[do][:], ps[:])
                            else:
                                qb = qbp.tile([P, QB], F32R, name="qb")
                                nc.scalar.copy(qb[:], ps[:])
                                nc.gpsimd.dma_start(
                                    out=qtspill[do], in_=qb[:]
                                )

            # ---------------- P2: attention per block ----------------
            with (
                tc.tile_pool(name="w2", bufs=1) as wp2,
                tc.tile_pool(name="et", bufs=1) as etp,
                tc.tile_pool(name="vld", bufs=4) as vlp,
                tc.tile_pool(name="cm", bufs=2) as cmp_,
                tc.tile_pool(name="ctxs", bufs=1) as ctp,
                tc.tile_pool(name="osb", bufs=2) as osp,
                tc.tile_pool(name="rd", bufs=1) as rdp,
            ):
                wo = load_w(wp2, "Wo")
                for blk in (0, 1):
                    nkc = NKC[blk]
                    cmbase = 0 if blk == 0 else NKC[0]
                    if blk == 1:
                        # reload block-B qT from DRAM into the shared qt slots
                        for do in range(NDC):
                            nc.gpsimd.dma_start(out=qt[do][:], in_=qtspill[do])
                    # S phase: scores^T -> exp -> mask
                    et = [etp.tile([P, QB], F32R, name=f"et{i}") for i in range(nkc)]
                    for kc in range(nkc):
                        ps = ps_s.tile([P, QB], F32, name="pss")
                        for di in range(NDC):
                            nc.tensor.matmul(
                                ps[:],
                                kt[di][:, kc * P:(kc + 1) * P],
                                qt[di][:],
                                start=(di == 0),
                                stop=(di == NDC - 1),
                            )
                        nc.scalar.activation(
                            et[kc][:], ps[:], mybir.ActivationFunctionType.Exp,
                            scale=SCALE,
                        )
                        cm = cmp_.tile([P, QB], F32, name="cm")
                        nc.gpsimd.dma_start(out=cm[:], in_=cm_d[cmbase + kc])
                        nc.vector.tensor_mul(et[kc][:], et[kc][:], cm[:])

                    # DEN phase: den_row[1,q] = ones^T @ e^T, recip, then
                    # PE-transpose each 128-q slice into [128,1] columns
                    d_row = rdp.tile([1, QB], F32, name=f"dr{blk}")
                    r_row = rdp.tile([1, QB], F32, name=f"rr{blk}")
                    r_t = rdp.tile([P, NQS], F32, name=f"rt{blk}")
                    psd = ps_o.tile([1, QB], F32, name="pso", tag="o")
                    for kc in range(nkc):
                        nc.tensor.matmul(
                            psd[:],
                            onesr[:],
                            et[kc][:],
                            start=(kc == 0),
                            stop=(kc == nkc - 1),
                        )
                    nc.vector.tensor_copy(d_row[:], psd[:])
                    nc.vector.reciprocal(r_row[:], d_row[:])
                    for qs in range(NQS):
                        nc.tensor.matmul(
                            scratch[:, 4 + qs:5 + qs],
                            r_row[0:1, qs * P:(qs + 1) * P],
                            ones_t[0:1, 0:1],
                            is_transpose=True,
                            start=True,
                            stop=True,
                        )
                    nc.vector.tensor_copy(r_t[:], scratch[:, 4:4 + NQS])

                    # PV phase: ctx^T[d, q] += v[k, d]^T-slices @ e^T[k, q]
                    ctxs = [
                        ctp.tile([P, QB], F32R, name=f"ctxs{i}") for i in range(NDC)
                    ]
                    for chunk in PV_PASSES:
                        w_pass = len(chunk) * P
                        psc = [ps_c.tile([P, QB], F32, name="psc") for _ in chunk]
                        for kc in range(nkc):
                            vl = vlp.tile([P, 3 * P], F32R, name="vld")
                            nc.gpsimd.dma_start(
                                out=vl[:, :w_pass],
                                in_=vspill[
                                    kc, :, chunk[0] * P:chunk[0] * P + w_pass
                                ],
                            )
                            for j, dc in enumerate(chunk):
                                nc.tensor.matmul(
                                    psc[j][:],
                                    vl[:, j * P:(j + 1) * P],
                                    et[kc][:],
                                    start=(kc == 0),
                                    stop=(kc == nkc - 1),
                                )
                        for j, dc in enumerate(chunk):
                            nc.vector.tensor_copy(ctxs[dc][:], psc[j][:])

                    # OPROJ phase: Z = ctx^T.T @ Wo, normalize, store
                    for qs in range(NQS):
                        for dh in range(2):
                            pso = ps_o.tile([P, QB], F32, name="pso", tag="o")
                            for dc in range(NDC):
                                nc.tensor.matmul(
                                    pso[:],
                                    ctxs[dc][:, qs * P:(qs + 1) * P],
                                    wo[:, dc, dh * QB:(dh + 1) * QB],
                                    start=(dc == 0),
                                    stop=(dc == NDC - 1),
                                )
                            ot = osp.tile([P, QB], F32, name="osb")
                            nc.vector.tensor_scalar_mul(
                                ot[:], pso[:], r_t[:, qs:qs + 1]
                            )
                            nc.gpsimd.dma_start(
                                out=out_d[
                                    blk * QB + qs * P: blk * QB + (qs + 1) * P,
                                    dh * QB:(dh + 1) * QB,
                                ],
                                in_=ot[:],
                            )
    nc.compile()
    return nc


_PROG = None


def _get_program():
    global _PROG
    if _PROG is None:
        _PROG = _build_program()
    return _PROG


def _make_core_inputs(x, Wq, Wk, Wv, Wo):
    """Build the per-core input maps (host-side sharding)."""
    in_maps = []
    qarr = np.arange(QB)
    for c in range(8):
        b, h = c // 2, c % 2
        xTb = np.ascontiguousarray(x[b].T)          # [D, S]
        q0A, q0B = Q_STARTS[h]
        qxT = np.ascontiguousarray(
            np.concatenate([x[b, q0A:q0A + QB], x[b, q0B:q0B + QB]], axis=0).T
        )                                            # [D, 2*QB]
        cm = np.empty((sum(NKC), P, QB), dtype=np.float32)
        for blk, (nkc, q0) in enumerate(zip(NKC, (q0A, q0B))):
            base = 0 if blk == 0 else NKC[0]
            for kc in range(nkc):
                karr = kc * P + np.arange(P)
                cm[base + kc] = (karr[:, None] <= (q0 + qarr)[None, :]).astype(
                    np.float32
                )
        in_maps.append(
            {
                "xT": xTb,
                "qxT": qxT,
                "Wq": Wq,
                "Wk": Wk,
                "Wv": Wv,
                "Wo": Wo,
                "cmask": cm,
            }
        )
    return in_maps


def _run(inputs, trace=False, trace_kwargs=None):
    x = np.asarray(inputs["x"], dtype=np.float32)
    Wq = np.asarray(inputs["Wq"], dtype=np.float32)
    Wk = np.asarray(inputs["Wk"], dtype=np.float32)
    Wv = np.asarray(inputs["Wv"], dtype=np.float32)
    Wo = np.asarray(inputs["Wo"], dtype=np.float32)
    bq = np.asarray(inputs["bq"], dtype=np.float32)
    bk = np.asarray(inputs["bk"], dtype=np.float32)
    bv = np.asarray(inputs["bv"], dtype=np.float32)
    bo = np.asarray(inputs["bo"], dtype=np.float32)
    assert not (np.any(bq) or np.any(bk)), "nonzero bq/bk unsupported"

    nc = _get_program()
    in_maps = _make_core_inputs(x, Wq, Wk, Wv, Wo)
    res = run_bass_kernel_spmd(
        nc, in_maps, list(range(8)), trace=trace, **(trace_kwargs or {})
    )

    out = np.empty((B, S, D), dtype=np.float32)
    for c in range(8):
        b, h = c // 2, c % 2
        q0A, q0B = Q_STARTS[h]
        o = res.results[c]["o_out"]
        out[b, q0A:q0A + QB] = o[:QB]
        out[b, q0B:q0B + QB] = o[QB:]
    out += bv @ Wo + bo                     # exact: attn rows sum to 1
    return out, res


def kernel(**inputs):
    out, _ = _run(inputs)
    return out


# revision 11
# speedup vs baseline: 1.3577x; 1.0748x over previous
"""Single-head causal attention (B=4, S=2048, D=1024) on 8 TRN2 NeuronCores.

Sharding: core c -> (batch b = c//2, half h = c%2). Each core computes the
full K/V projections for its batch and attends two 512-query blocks chosen
so causal work balances across the two cores of a batch:
  h=0: query rows [0:512)    and [1536:2048)   (4 + 16 causal key-chunks)
  h=1: query rows [512:1024) and [1024:1536)   (8 + 12 causal key-chunks)
The SPMD program is uniform: block A always scans 8 key-chunks, block B 16;
out-of-causal-range chunks are zeroed by a host-supplied multiplicative mask
(which also applies the intra-diagonal triangle), so all 8 cores run the
same instruction stream on different data.

Layout: everything transposed. xT/qT/kT are [d_part, seq_free]; scores are
computed as S^T [key_part, q_free] so exp runs on ScalarE along the free
axis with no transposes anywhere. Softmax uses no max-subtraction (scores
are O(few) by construction), and normalization is deferred: unnormalized
ctx flows through the output projection and each [128q, dout] result tile
is scaled by 1/denom as a per-partition scalar. Denominators come from N=1
matmuls vs a ones vector. Biases are handled on the host: bq/bk are
exactly zero in this problem, and bv/bo enter additively as (bv @ Wo + bo).

Matmuls run as float32r (full-rate fp32 on the PE at N>=256). The fused
fp32r matmul ISA slot carries at most ONE sync wait, so the program is
arranged so every matmul has at most one uncovered dependency:
  - every logical load is a single DMA instruction (one DMA-lane dep),
  - tiny "absorber" matmuls into a dedicated scratch PSUM bank observe
    each DMA lane on the PE before the real matmul group needs it,
  - PSUM->SBUF copies are routed per-phase to a single engine so psum-WAR
    and operand deps collapse into one engine-tick wait.
"""

import numpy as np

import concourse.bass as bass
import concourse.bacc as bacc
import concourse.mybir as mybir
from concourse.tile import TileContext
from concourse.bass_utils import run_bass_kernel_spmd

B, S, D = 4, 2048, 1024
P = 128
QB = 512                    # query-block width (free dim of score matmuls)
NKC = (8, 16)               # key-chunks scanned for block A / block B
NDC = D // P                # 8 d-chunks
NKB = S // QB               # 4 key-blocks in projection
NQS = QB // P               # 4 query sub-tiles per block
PV_PASSES = ((0, 1, 2), (3, 4, 5), (6, 7))
F32 = mybir.dt.float32
F32R = mybir.dt.float32r
SCALE = 1.0 / float(np.sqrt(D))

# q-row starts per (h, block)
Q_STARTS = {0: (0, 3 * QB), 1: (QB, 2 * QB)}


def _r(ap):
    return ap.bitcast(mybir.dt.float32r)


def _build_program():
    nc = bacc.Bacc("TRN2", target_bir_lowering=False, debug=False)
    xT = nc.declare_dram_parameter("xT", [D, S], F32, isOutput=False)
    qxT = nc.declare_dram_parameter("qxT", [D, 2 * QB], F32, isOutput=False)
    w_d = {
        n: nc.declare_dram_parameter(n, [D, D], F32, isOutput=False)
        for n in ("Wq", "Wk", "Wv", "Wo")
    }
    cm_d = nc.declare_dram_parameter("cmask", [sum(NKC), P, QB], F32, isOutput=False)
    out_d = nc.declare_dram_parameter("o_out", [2 * QB, D], F32, isOutput=True)
    vspill = nc.dram_tensor("vspill", [S // P, P, D], F32R)
    qtspill = nc.dram_tensor("qtspill", [NDC, P, QB], F32R)

    with TileContext(nc) as tc:
        with (
            tc.tile_pool(name="persist", bufs=1) as pp,
            tc.tile_pool(name="ps_s", bufs=2, space="PSUM") as ps_s,
            tc.tile_pool(name="ps_c", bufs=3, space="PSUM") as ps_c,
            tc.tile_pool(name="ps_o", bufs=2, space="PSUM") as ps_o,
            tc.tile_pool(name="ps_x", bufs=1, space="PSUM") as ps_x,
        ):
            # persistent SBUF tensors
            kt = [pp.tile([P, S], F32R, name=f"kt{i}") for i in range(NDC)]
            qt = [pp.tile([P, QB], F32R, name=f"qt{i}") for i in range(NDC)]
            ones_t = pp.tile([P, 2], F32, name="ones_t")
            nc.vector.memset(ones_t[:], 1.0)
            onesr = pp.tile([P, 1], F32R, name="onesr")
            nc.scalar.copy(onesr[:], ones_t[:, 0:1])

            scratch = ps_x.tile([P, 16], F32, name="scratch")

            def load_w(pool, wname, nchunk=4):
                wall = pool.tile([P, NDC, D], F32R, name="wall")
                wsrc = _r(w_d[wname].rearrange("(a p) d -> p a d", p=P))
                step = NDC // nchunk
                for c in range(nchunk):
                    nc.sync.dma_start(
                        out=wall[:, c * step:(c + 1) * step, :],
                        in_=wsrc[:, c * step:(c + 1) * step, :],
                    )
                return wall

            # ---------------- P1: projections ----------------
            with (
                tc.tile_pool(name="w", bufs=2) as wp,
                tc.tile_pool(name="xtk", bufs=2) as xtp,
            ):

                def load_xt(src, col0):
                    xta = xtp.tile([P, NDC, QB], F32R, name="xta")
                    xsrc = _r(
                        src.rearrange("(a p) s -> p a s", p=P)[
                            :, :, col0:col0 + QB
                        ]
                    )
                    half = NDC // 2
                    for c in range(2):
                        nc.sync.dma_start(
                            out=xta[:, c * half:(c + 1) * half, :],
                            in_=xsrc[:, c * half:(c + 1) * half, :],
                        )
                    return xta

                # -- rounds 1+2 merged: kT and V from one pass over x^T
                wk = load_w(wp, "Wk")
                wv = load_w(wp, "Wv")
                with tc.tile_pool(name="vtmp", bufs=3) as vtp:
                    for kb in range(NKB):
                        xta = load_xt(xT, kb * QB)
                        for do in range(NDC):
                            ps = ps_s.tile([P, QB], F32, name="pss")
                            for di in range(NDC):
                                nc.tensor.matmul(
                                    ps[:],
                                    wk[:, di, do * P:(do + 1) * P],
                                    xta[:, di, :],
                                    start=(di == 0),
                                    stop=(di == NDC - 1),
                                )
                            nc.scalar.copy(kt[do][:, kb * QB:(kb + 1) * QB], ps[:])
                        for kcl in range(QB // P):
                            kc = kb * (QB // P) + kcl
                            for dh in range(2):
                                ps = ps_s.tile([P, QB], F32, name="pss")
                                for di in range(NDC):
                                    nc.tensor.matmul(
                                        ps[:],
                                        xta[:, di, kcl * P:(kcl + 1) * P],
                                        wv[:, di, dh * QB:(dh + 1) * QB],
                                        start=(di == 0),
                                        stop=(di == NDC - 1),
                                    )
                                vt = vtp.tile([P, QB], F32R, name="vtmp")
                                nc.vector.tensor_copy(vt[:], ps[:])
                                nc.sync.dma_start(
                                    out=vspill[kc, :, dh * QB:(dh + 1) * QB],
                                    in_=vt[:],
                                )

                # -- round 3: qT = Wq^T x^T; block A kept in SBUF, block B
                #    bounced through DRAM (frees 16KB for W double-buffering)
                wq = load_w(wp, "Wq")
                with tc.tile_pool(name="qb", bufs=3) as qbp:
                    for blk in (0, 1):
                        xta = load_xt(qxT, blk * QB)
                        for do in range(NDC):
                            ps = ps_s.tile([P, QB], F32, name="pss")
                            for di in range(NDC):
                                nc.tensor.matmul(
                                    ps[:],
                                    wq[:, di, do * P:(do + 1) * P],
                                    xta[:, di, :],
                                    start=(di == 0),
                                    stop=(di == NDC - 1),
                                )
                            if blk == 0:
                                nc.scalar.copy(qt[do][:], ps[:])
                            else:
                                qb = qbp.tile([P, QB], F32R, name="qb")
                                nc.scalar.copy(qb[:], ps[:])
                                nc.sync.dma_start(
                                    out=qtspill[do], in_=qb[:]
                                )

            # ---------------- P2: attention per block ----------------
            with (
                tc.tile_pool(name="w2", bufs=1) as wp2,
                tc.tile_pool(name="et", bufs=1) as etp,
                tc.tile_pool(name="vld", bufs=4) as vlp,
                tc.tile_pool(name="cm", bufs=2) as cmp_,
                tc.tile_pool(name="ctxs", bufs=1) as ctp,
                tc.tile_pool(name="osb", bufs=2) as osp,
                tc.tile_pool(name="rd", bufs=1) as rdp,
            ):
                wo = load_w(wp2, "Wo")
                for blk in (0, 1):
                    nkc = NKC[blk]
                    cmbase = 0 if blk == 0 else NKC[0]
                    if blk == 1:
                        # reload block-B qT from DRAM into the shared qt slots
                        for do in range(NDC):
                            nc.sync.dma_start(out=qt[do][:], in_=qtspill[do])
                    # S phase: scores^T -> exp -> mask
                    et = [etp.tile([P, QB], F32R, name=f"et{i}") for i in range(nkc)]
                    for kc in range(nkc):
                        ps = ps_s.tile([P, QB], F32, name="pss")
                        for di in range(NDC):
                            nc.tensor.matmul(
                                ps[:],
                                kt[di][:, kc * P:(kc + 1) * P],
                                qt[di][:],
                                start=(di == 0),
                                stop=(di == NDC - 1),
                            )
                        nc.scalar.activation(
                            et[kc][:], ps[:], mybir.ActivationFunctionType.Exp,
                            scale=SCALE,
                        )
                        if blk == 0 or kc >= 8:
                            cm = cmp_.tile([P, QB], F32, name="cm")
                            nc.sync.dma_start(out=cm[:], in_=cm_d[cmbase + kc])
                            nc.vector.tensor_mul(et[kc][:], et[kc][:], cm[:])

                    # DEN phase: den_row[1,q] = ones^T @ e^T, recip, then
                    # PE-transpose each 128-q slice into [128,1] columns
                    d_row = rdp.tile([1, QB], F32, name=f"dr{blk}")
                    r_row = rdp.tile([1, QB], F32, name=f"rr{blk}")
                    r_t = rdp.tile([P, NQS], F32, name=f"rt{blk}")
                    psd = ps_o.tile([1, QB], F32, name="pso", tag="o")
                    for kc in range(nkc):
                        nc.tensor.matmul(
                            psd[:],
                            onesr[:],
                            et[kc][:],
                            start=(kc == 0),
                            stop=(kc == nkc - 1),
                        )
                    nc.vector.tensor_copy(d_row[:], psd[:])
                    nc.vector.reciprocal(r_row[:], d_row[:])
                    for qs in range(NQS):
                        nc.tensor.matmul(
                            scratch[:, 4 + qs:5 + qs],
                            r_row[0:1, qs * P:(qs + 1) * P],
                            ones_t[0:1, 0:1],
                            is_transpose=True,
                            start=True,
                            stop=True,
                        )
                    nc.vector.tensor_copy(r_t[:], scratch[:, 4:4 + NQS])

                    # PV phase: ctx^T[d, q] += v[k, d]^T-slices @ e^T[k, q]
                    ctxs = [
                        ctp.tile([P, QB], F32R, name=f"ctxs{i}") for i in range(NDC)
                    ]
                    for chunk in PV_PASSES:
                        w_pass = len(chunk) * P
                        psc = [ps_c.tile([P, QB], F32, name="psc") for _ in chunk]
                        for kc in range(nkc):
                            vl = vlp.tile([P, 3 * P], F32R, name="vld")
                            nc.sync.dma_start(
                                out=vl[:, :w_pass],
                                in_=vspill[
                                    kc, :, chunk[0] * P:chunk[0] * P + w_pass
                                ],
                            )
                            for j, dc in enumerate(chunk):
                                nc.tensor.matmul(
                                    psc[j][:],
                                    vl[:, j * P:(j + 1) * P],
                                    et[kc][:],
                                    start=(kc == 0),
                                    stop=(kc == nkc - 1),
                                )
                        for j, dc in enumerate(chunk):
                            nc.vector.tensor_copy(ctxs[dc][:], psc[j][:])

                    # OPROJ phase: Z = ctx^T.T @ Wo, normalize, store
                    for qs in range(NQS):
                        for dh in range(2):
                            pso = ps_o.tile([P, QB], F32, name="pso", tag="o")
                            for dc in range(NDC):
                                nc.tensor.matmul(
                                    pso[:],
                                    ctxs[dc][:, qs * P:(qs + 1) * P],
                                    wo[:, dc, dh * QB:(dh + 1) * QB],
                                    start=(dc == 0),
                                    stop=(dc == NDC - 1),
                                )
                            ot = osp.tile([P, QB], F32, name="osb")
                            nc.vector.tensor_scalar_mul(
                                ot[:], pso[:], r_t[:, qs:qs + 1]
                            )
                            nc.sync.dma_start(
                                out=out_d[
                                    blk * QB + qs * P: blk * QB + (qs + 1) * P,
                                    dh * QB:(dh + 1) * QB,
                                ],
                                in_=ot[:],
                            )
    nc.compile()
    return nc


_PROG = None


def _get_program():
    global _PROG
    if _PROG is None:
        _PROG = _build_program()
    return _PROG


def _make_core_inputs(x, Wq, Wk, Wv, Wo):
    """Build the per-core input maps (host-side sharding)."""
    in_maps = []
    qarr = np.arange(QB)
    for c in range(8):
        b, h = c // 2, c % 2
        xTb = np.ascontiguousarray(x[b].T)          # [D, S]
        q0A, q0B = Q_STARTS[h]
        qxT = np.ascontiguousarray(
            np.concatenate([x[b, q0A:q0A + QB], x[b, q0B:q0B + QB]], axis=0).T
        )                                            # [D, 2*QB]
        cm = np.empty((sum(NKC), P, QB), dtype=np.float32)
        for blk, (nkc, q0) in enumerate(zip(NKC, (q0A, q0B))):
            base = 0 if blk == 0 else NKC[0]
            for kc in range(nkc):
                karr = kc * P + np.arange(P)
                cm[base + kc] = (karr[:, None] <= (q0 + qarr)[None, :]).astype(
                    np.float32
                )
        in_maps.append(
            {
                "xT": xTb,
                "qxT": qxT,
                "Wq": Wq,
                "Wk": Wk,
                "Wv": Wv,
                "Wo": Wo,
                "cmask": cm,
            }
        )
    return in_maps


def _run(inputs, trace=False, trace_kwargs=None):
    x = np.asarray(inputs["x"], dtype=np.float32)
    Wq = np.asarray(inputs["Wq"], dtype=np.float32)
    Wk = np.asarray(inputs["Wk"], dtype=np.float32)
    Wv = np.asarray(inputs["Wv"], dtype=np.float32)
    Wo = np.asarray(inputs["Wo"], dtype=np.float32)
    bq = np.asarray(inputs["bq"], dtype=np.float32)
    bk = np.asarray(inputs["bk"], dtype=np.float32)
    bv = np.asarray(inputs["bv"], dtype=np.float32)
    bo = np.asarray(inputs["bo"], dtype=np.float32)
    assert not (np.any(bq) or np.any(bk)), "nonzero bq/bk unsupported"

    nc = _get_program()
    in_maps = _make_core_inputs(x, Wq, Wk, Wv, Wo)
    res = run_bass_kernel_spmd(
        nc, in_maps, list(range(8)), trace=trace, **(trace_kwargs or {})
    )

    out = np.empty((B, S, D), dtype=np.float32)
    for c in range(8):
        b, h = c // 2, c % 2
        q0A, q0B = Q_STARTS[h]
        o = res.results[c]["o_out"]
        out[b, q0A:q0A + QB] = o[:QB]
        out[b, q0B:q0B + QB] = o[QB:]
    out += bv @ Wo + bo                     # exact: attn rows sum to 1
    return out, res


def kernel(**inputs):
    out, _ = _run(inputs)
    return out


# revision 12
# speedup vs baseline: 1.3890x; 1.0231x over previous
"""Single-head causal attention (B=4, S=2048, D=1024) on 8 TRN2 NeuronCores.

Sharding: core c -> (batch b = c//2, half h = c%2). Each core computes the
full K/V projections for its batch and attends two 512-query blocks chosen
so causal work balances across the two cores of a batch:
  h=0: query rows [0:512)    and [1536:2048)   (4 + 16 causal key-chunks)
  h=1: query rows [512:1024) and [1024:1536)   (8 + 12 causal key-chunks)
The SPMD program is uniform: block A always scans 8 key-chunks, block B 16;
out-of-causal-range chunks are zeroed by a host-supplied multiplicative mask
(which also applies the intra-diagonal triangle), so all 8 cores run the
same instruction stream on different data.

Layout: everything transposed. xT/qT/kT are [d_part, seq_free]; scores are
computed as S^T [key_part, q_free] so exp runs on ScalarE along the free
axis with no transposes anywhere. Softmax uses no max-subtraction (scores
are O(few) by construction), and normalization is deferred: unnormalized
ctx flows through the output projection and each [128q, dout] result tile
is scaled by 1/denom as a per-partition scalar. Denominators come from N=1
matmuls vs a ones vector. Biases are handled on the host: bq/bk are
exactly zero in this problem, and bv/bo enter additively as (bv @ Wo + bo).

Matmuls run as float32r (full-rate fp32 on the PE at N>=256). The fused
fp32r matmul ISA slot carries at most ONE sync wait, so the program is
arranged so every matmul has at most one uncovered dependency:
  - every logical load is a single DMA instruction (one DMA-lane dep),
  - tiny "absorber" matmuls into a dedicated scratch PSUM bank observe
    each DMA lane on the PE before the real matmul group needs it,
  - PSUM->SBUF copies are routed per-phase to a single engine so psum-WAR
    and operand deps collapse into one engine-tick wait.
"""

import numpy as np

import concourse.bass as bass
import concourse.bacc as bacc
import concourse.mybir as mybir
from concourse.tile import TileContext
from concourse.bass_utils import run_bass_kernel_spmd

B, S, D = 4, 2048, 1024
P = 128
QB = 512                    # query-block width (free dim of score matmuls)
NKC = (8, 16)               # key-chunks scanned for block A / block B
NDC = D // P                # 8 d-chunks
NKB = S // QB               # 4 key-blocks in projection
NQS = QB // P               # 4 query sub-tiles per block
PV_PASSES = ((0, 1, 2), (3, 4, 5), (6, 7))
F32 = mybir.dt.float32
F32R = mybir.dt.float32r
SCALE = 1.0 / float(np.sqrt(D))

# q-row starts per (h, block)
Q_STARTS = {0: (0, 3 * QB), 1: (QB, 2 * QB)}


def _r(ap):
    return ap.bitcast(mybir.dt.float32r)


def _build_program():
    nc = bacc.Bacc("TRN2", target_bir_lowering=False, debug=False)
    xT = nc.declare_dram_parameter("xT", [D, S], F32, isOutput=False)
    qxT = nc.declare_dram_parameter("qxT", [D, 2 * QB], F32, isOutput=False)
    w_d = {
        n: nc.declare_dram_parameter(n, [D, D], F32, isOutput=False)
        for n in ("Wq", "Wk", "Wv", "Wo")
    }
    cm_d = nc.declare_dram_parameter("cmask", [sum(NKC), P, QB], F32, isOutput=False)
    out_d = nc.declare_dram_parameter("o_out", [2 * QB, D], F32, isOutput=True)
    vspill = nc.dram_tensor("vspill", [S // P, P, D], F32R)
    qtspill = nc.dram_tensor("qtspill", [NDC, P, QB], F32R)

    with TileContext(nc) as tc:
        with (
            tc.tile_pool(name="persist", bufs=1) as pp,
            tc.tile_pool(name="ps_s", bufs=2, space="PSUM") as ps_s,
            tc.tile_pool(name="ps_c", bufs=3, space="PSUM") as ps_c,
            tc.tile_pool(name="ps_o", bufs=2, space="PSUM") as ps_o,
            tc.tile_pool(name="ps_x", bufs=1, space="PSUM") as ps_x,
        ):
            # persistent SBUF tensors
            kt = [pp.tile([P, S], F32R, name=f"kt{i}") for i in range(NDC)]
            qt = [pp.tile([P, QB], F32R, name=f"qt{i}") for i in range(NDC)]
            ones_t = pp.tile([P, 2], F32, name="ones_t")
            nc.vector.memset(ones_t[:], 1.0)
            onesr = pp.tile([P, 1], F32R, name="onesr")
            nc.scalar.copy(onesr[:], ones_t[:, 0:1])

            scratch = ps_x.tile([P, 16], F32, name="scratch")

            def load_w(pool, wname, nchunk=4):
                wall = pool.tile([P, NDC, D], F32R, name="wall")
                wsrc = _r(w_d[wname].rearrange("(a p) d -> p a d", p=P))
                step = NDC // nchunk
                for c in range(nchunk):
                    nc.sync.dma_start(
                        out=wall[:, c * step:(c + 1) * step, :],
                        in_=wsrc[:, c * step:(c + 1) * step, :],
                    )
                return wall

            # ---------------- P1: projections ----------------
            with (
                tc.tile_pool(name="w", bufs=2) as wp,
                tc.tile_pool(name="xtk", bufs=2) as xtp,
            ):

                def load_xt(src, col0):
                    xta = xtp.tile([P, NDC, QB], F32R, name="xta")
                    xsrc = _r(
                        src.rearrange("(a p) s -> p a s", p=P)[
                            :, :, col0:col0 + QB
                        ]
                    )
                    half = NDC // 2
                    for c in range(2):
                        nc.sync.dma_start(
                            out=xta[:, c * half:(c + 1) * half, :],
                            in_=xsrc[:, c * half:(c + 1) * half, :],
                        )
                    return xta

                # -- rounds 1+2 merged: kT and V from one pass over x^T.
                # Build order sets DMA priority: xta(kb=0) and Wk first so
                # the PE starts ASAP; Wv is issued after the first kT
                # groups and loads under their compute.
                xta = load_xt(xT, 0)
                wk = load_w(wp, "Wk")
                wv = None
                with tc.tile_pool(name="vtmp", bufs=3) as vtp:
                    for kb in range(NKB):
                        if kb > 0:
                            xta = load_xt(xT, kb * QB)
                        for do in range(NDC):
                            ps = ps_s.tile([P, QB], F32, name="pss")
                            for di in range(NDC):
                                nc.tensor.matmul(
                                    ps[:],
                                    wk[:, di, do * P:(do + 1) * P],
                                    xta[:, di, :],
                                    start=(di == 0),
                                    stop=(di == NDC - 1),
                                )
                            nc.scalar.copy(kt[do][:, kb * QB:(kb + 1) * QB], ps[:])
                        if wv is None:
                            wv = load_w(wp, "Wv")
                        for kcl in range(QB // P):
                            kc = kb * (QB // P) + kcl
                            for dh in range(2):
                                ps = ps_s.tile([P, QB], F32, name="pss")
                                for di in range(NDC):
                                    nc.tensor.matmul(
                                        ps[:],
                                        xta[:, di, kcl * P:(kcl + 1) * P],
                                        wv[:, di, dh * QB:(dh + 1) * QB],
                                        start=(di == 0),
                                        stop=(di == NDC - 1),
                                    )
                                vt = vtp.tile([P, QB], F32R, name="vtmp")
                                nc.vector.tensor_copy(vt[:], ps[:])
                                nc.sync.dma_start(
                                    out=vspill[kc, :, dh * QB:(dh + 1) * QB],
                                    in_=vt[:],
                                )

                # -- round 3: qT = Wq^T x^T; block A kept in SBUF, block B
                #    bounced through DRAM (frees 16KB for W double-buffering)
                wq = load_w(wp, "Wq")
                with tc.tile_pool(name="qb", bufs=3) as qbp:
                    for blk in (0, 1):
                        xta = load_xt(qxT, blk * QB)
                        for do in range(NDC):
                            ps = ps_s.tile([P, QB], F32, name="pss")
                            for di in range(NDC):
                                nc.tensor.matmul(
                                    ps[:],
                                    wq[:, di, do * P:(do + 1) * P],
                                    xta[:, di, :],
                                    start=(di == 0),
                                    stop=(di == NDC - 1),
                                )
                            if blk == 0:
                                nc.scalar.copy(qt[do][:], ps[:])
                            else:
                                qb = qbp.tile([P, QB], F32R, name="qb")
                                nc.scalar.copy(qb[:], ps[:])
                                nc.sync.dma_start(
                                    out=qtspill[do], in_=qb[:]
                                )

            # ---------------- P2: attention per block ----------------
            with (
                tc.tile_pool(name="w2", bufs=1) as wp2,
                tc.tile_pool(name="et", bufs=1) as etp,
                tc.tile_pool(name="vld", bufs=4) as vlp,
                tc.tile_pool(name="cm", bufs=2) as cmp_,
                tc.tile_pool(name="ctxs", bufs=1) as ctp,
                tc.tile_pool(name="osb", bufs=2) as osp,
                tc.tile_pool(name="rd", bufs=1) as rdp,
            ):
                wo = load_w(wp2, "Wo")
                for blk in (0, 1):
                    nkc = NKC[blk]
                    cmbase = 0 if blk == 0 else NKC[0]
                    if blk == 1:
                        # reload block-B qT from DRAM into the shared qt slots
                        for do in range(NDC):
                            nc.sync.dma_start(out=qt[do][:], in_=qtspill[do])
                    # S phase: scores^T -> exp -> mask
                    et = [etp.tile([P, QB], F32R, name=f"et{i}") for i in range(nkc)]
                    for kc in range(nkc):
                        ps = ps_s.tile([P, QB], F32, name="pss")
                        for di in range(NDC):
                            nc.tensor.matmul(
                                ps[:],
                                kt[di][:, kc * P:(kc + 1) * P],
                                qt[di][:],
                                start=(di == 0),
                                stop=(di == NDC - 1),
                            )
                        nc.scalar.activation(
                            et[kc][:], ps[:], mybir.ActivationFunctionType.Exp,
                            scale=SCALE,
                        )
                        if blk == 0 or kc >= 8:
                            cm = cmp_.tile([P, QB], F32, name="cm")
                            nc.sync.dma_start(out=cm[:], in_=cm_d[cmbase + kc])
                            nc.vector.tensor_mul(et[kc][:], et[kc][:], cm[:])

                    # DEN phase: den_row[1,q] = ones^T @ e^T, recip, then
                    # PE-transpose each 128-q slice into [128,1] columns
                    d_row = rdp.tile([1, QB], F32, name=f"dr{blk}")
                    r_row = rdp.tile([1, QB], F32, name=f"rr{blk}")
                    r_t = rdp.tile([P, NQS], F32, name=f"rt{blk}")
                    psd = ps_o.tile([1, QB], F32, name="pso", tag="o")
                    for kc in range(nkc):
                        nc.tensor.matmul(
                            psd[:],
                            onesr[:],
                            et[kc][:],
                            start=(kc == 0),
                            stop=(kc == nkc - 1),
                        )
                    nc.vector.tensor_copy(d_row[:], psd[:])
                    nc.vector.reciprocal(r_row[:], d_row[:])
                    for qs in range(NQS):
                        nc.tensor.matmul(
                            scratch[:, 4 + qs:5 + qs],
                            r_row[0:1, qs * P:(qs + 1) * P],
                            ones_t[0:1, 0:1],
                            is_transpose=True,
                            start=True,
                            stop=True,
                        )
                    nc.vector.tensor_copy(r_t[:], scratch[:, 4:4 + NQS])

                    # PV phase: ctx^T[d, q] += v[k, d]^T-slices @ e^T[k, q]
                    ctxs = [
                        ctp.tile([P, QB], F32R, name=f"ctxs{i}") for i in range(NDC)
                    ]
                    for chunk in PV_PASSES:
                        w_pass = len(chunk) * P
                        psc = [ps_c.tile([P, QB], F32, name="psc") for _ in chunk]
                        for kc in range(nkc):
                            vl = vlp.tile([P, 3 * P], F32R, name="vld")
                            nc.sync.dma_start(
                                out=vl[:, :w_pass],
                                in_=vspill[
                                    kc, :, chunk[0] * P:chunk[0] * P + w_pass
                                ],
                            )
                            for j, dc in enumerate(chunk):
                                nc.tensor.matmul(
                                    psc[j][:],
                                    vl[:, j * P:(j + 1) * P],
                                    et[kc][:],
                                    start=(kc == 0),
                                    stop=(kc == nkc - 1),
                                )
                        for j, dc in enumerate(chunk):
                            nc.vector.tensor_copy(ctxs[dc][:], psc[j][:])

                    # OPROJ phase: Z = ctx^T.T @ Wo, normalize, store
                    for qs in range(NQS):
                        for dh in range(2):
                            pso = ps_o.tile([P, QB], F32, name="pso", tag="o")
                            for dc in range(NDC):
                                nc.tensor.matmul(
                                    pso[:],
                                    ctxs[dc][:, qs * P:(qs + 1) * P],
                                    wo[:, dc, dh * QB:(dh + 1) * QB],
                                    start=(dc == 0),
                                    stop=(dc == NDC - 1),
                                )
                            ot = osp.tile([P, QB], F32, name="osb")
                            nc.vector.tensor_scalar_mul(
                                ot[:], pso[:], r_t[:, qs:qs + 1]
                            )
                            nc.sync.dma_start(
                                out=out_d[
                                    blk * QB + qs * P: blk * QB + (qs + 1) * P,
                                    dh * QB:(dh + 1) * QB,
                                ],
                                in_=ot[:],
                            )
    nc.compile()
    return nc


_PROG = None


def _get_program():
    global _PROG
    if _PROG is None:
        _PROG = _build_program()
    return _PROG


def _make_core_inputs(x, Wq, Wk, Wv, Wo):
    """Build the per-core input maps (host-side sharding)."""
    in_maps = []
    qarr = np.arange(QB)
    for c in range(8):
        b, h = c // 2, c % 2
        xTb = np.ascontiguousarray(x[b].T)          # [D, S]
        q0A, q0B = Q_STARTS[h]
        qxT = np.ascontiguousarray(
            np.concatenate([x[b, q0A:q0A + QB], x[b, q0B:q0B + QB]], axis=0).T
        )                                            # [D, 2*QB]
        cm = np.empty((sum(NKC), P, QB), dtype=np.float32)
        for blk, (nkc, q0) in enumerate(zip(NKC, (q0A, q0B))):
            base = 0 if blk == 0 else NKC[0]
            for kc in range(nkc):
                karr = kc * P + np.arange(P)
                cm[base + kc] = (karr[:, None] <= (q0 + qarr)[None, :]).astype(
                    np.float32
                )
        in_maps.append(
            {
                "xT": xTb,
                "qxT": qxT,
                "Wq": Wq,
                "Wk": Wk,
                "Wv": Wv,
                "Wo": Wo,
                "cmask": cm,
            }
        )
    return in_maps


def _run(inputs, trace=False, trace_kwargs=None):
    x = np.asarray(inputs["x"], dtype=np.float32)
    Wq = np.asarray(inputs["Wq"], dtype=np.float32)
    Wk = np.asarray(inputs["Wk"], dtype=np.float32)
    Wv = np.asarray(inputs["Wv"], dtype=np.float32)
    Wo = np.asarray(inputs["Wo"], dtype=np.float32)
    bq = np.asarray(inputs["bq"], dtype=np.float32)
    bk = np.asarray(inputs["bk"], dtype=np.float32)
    bv = np.asarray(inputs["bv"], dtype=np.float32)
    bo = np.asarray(inputs["bo"], dtype=np.float32)
    assert not (np.any(bq) or np.any(bk)), "nonzero bq/bk unsupported"

    nc = _get_program()
    in_maps = _make_core_inputs(x, Wq, Wk, Wv, Wo)
    res = run_bass_kernel_spmd(
        nc, in_maps, list(range(8)), trace=trace, **(trace_kwargs or {})
    )

    out = np.empty((B, S, D), dtype=np.float32)
    for c in range(8):
        b, h = c // 2, c % 2
        q0A, q0B = Q_STARTS[h]
        o = res.results[c]["o_out"]
        out[b, q0A:q0A + QB] = o[:QB]
        out[b, q0B:q0B + QB] = o[QB:]
    out += bv @ Wo + bo                     # exact: attn rows sum to 1
    return out, res


def kernel(**inputs):
    out, _ = _run(inputs)
    return out


# revision 13
# speedup vs baseline: 1.6449x; 1.1843x over previous
"""Single-head causal attention (B=4, S=2048, D=1024) on 8 TRN2 NeuronCores.

Sharding: core c -> (batch b = c//2, half h = c%2). Each core computes the
full K/V projections for its batch and attends two 512-query blocks chosen
so causal work balances across the two cores of a batch:
  h=0: query rows [0:512)    and [1536:2048)   (4 + 16 causal key-chunks)
  h=1: query rows [512:1024) and [1024:1536)   (8 + 12 causal key-chunks)
The SPMD program is uniform: block A always scans 8 key-chunks, block B 16;
out-of-causal-range chunks are zeroed by a host-supplied multiplicative mask
(which also applies the intra-diagonal triangle), so all 8 cores run the
same instruction stream on different data.

Layout: everything transposed. xT/qT/kT are [d_part, seq_free]; scores are
computed as S^T [key_part, q_free] so exp runs on ScalarE along the free
axis with no transposes anywhere. Softmax uses no max-subtraction (scores
are O(few) by construction), and normalization is deferred: unnormalized
ctx flows through the output projection and each [128q, dout] result tile
is scaled by 1/denom as a per-partition scalar. Denominators come from N=1
matmuls vs a ones vector. Biases are handled on the host: bq/bk are
exactly zero in this problem, and bv/bo enter additively as (bv @ Wo + bo).

Matmuls run as float32r (full-rate fp32 on the PE at N>=256). The fused
fp32r matmul ISA slot carries at most ONE sync wait, so the program is
arranged so every matmul has at most one uncovered dependency:
  - every logical load is a single DMA instruction (one DMA-lane dep),
  - tiny "absorber" matmuls into a dedicated scratch PSUM bank observe
    each DMA lane on the PE before the real matmul group needs it,
  - PSUM->SBUF copies are routed per-phase to a single engine so psum-WAR
    and operand deps collapse into one engine-tick wait.
"""

import numpy as np

import concourse.bass as bass
import concourse.bacc as bacc
import concourse.mybir as mybir
from concourse.tile import TileContext
from concourse.bass_utils import run_bass_kernel_spmd

B, S, D = 4, 2048, 1024
P = 128
QB = 512                    # query-block width (free dim of score matmuls)
NKC = (8, 16)               # key-chunks scanned for block A / block B
NDC = D // P                # 8 d-chunks
NKB = S // QB               # 4 key-blocks in projection
NQS = QB // P               # 4 query sub-tiles per block
PV_PASSES = ((0, 1, 2), (3, 4, 5), (6, 7))
F32 = mybir.dt.float32
F32R = mybir.dt.float32r
SCALE = 1.0 / float(np.sqrt(D))

# q-row starts per (h, block)
Q_STARTS = {0: (0, 3 * QB), 1: (QB, 2 * QB)}


def _r(ap):
    return ap.bitcast(mybir.dt.float32r)


def _build_program():
    nc = bacc.Bacc("TRN2", target_bir_lowering=False, debug=False)
    xT = nc.declare_dram_parameter("xT", [D, S], F32, isOutput=False)
    qxT = nc.declare_dram_parameter("qxT", [D, 2 * QB], F32, isOutput=False)
    w_d = {
        n: nc.declare_dram_parameter(n, [D, D], F32, isOutput=False)
        for n in ("Wq", "Wk", "Wvo")
    }
    xnat = nc.declare_dram_parameter("xnat", [S, D], F32, isOutput=False)
    cm_d = nc.declare_dram_parameter("cmask", [sum(NKC), P, QB], F32, isOutput=False)
    out_d = nc.declare_dram_parameter("o_out", [2 * QB, D], F32, isOutput=True)
    qtspill = nc.dram_tensor("qtspill", [NDC, P, QB], F32R)

    with TileContext(nc) as tc:
        with (
            tc.tile_pool(name="persist", bufs=1) as pp,
            tc.tile_pool(name="ps_s", bufs=2, space="PSUM") as ps_s,
            tc.tile_pool(name="ps_c", bufs=3, space="PSUM") as ps_c,
            tc.tile_pool(name="ps_o", bufs=2, space="PSUM") as ps_o,
            tc.tile_pool(name="ps_x", bufs=1, space="PSUM") as ps_x,
        ):
            # persistent SBUF tensors
            kt = [pp.tile([P, S], F32R, name=f"kt{i}") for i in range(NDC)]
            qt = [pp.tile([P, QB], F32R, name=f"qt{i}") for i in range(NDC)]
            ones_t = pp.tile([P, 2], F32, name="ones_t")
            nc.vector.memset(ones_t[:], 1.0)
            onesr = pp.tile([P, 1], F32R, name="onesr")
            nc.scalar.copy(onesr[:], ones_t[:, 0:1])

            scratch = ps_x.tile([P, 16], F32, name="scratch")

            def load_w(pool, wname, nchunk=4):
                wall = pool.tile([P, NDC, D], F32R, name="wall")
                wsrc = _r(w_d[wname].rearrange("(a p) d -> p a d", p=P))
                step = NDC // nchunk
                for c in range(nchunk):
                    nc.sync.dma_start(
                        out=wall[:, c * step:(c + 1) * step, :],
                        in_=wsrc[:, c * step:(c + 1) * step, :],
                    )
                return wall

            # ---------------- P1: projections ----------------
            with (
                tc.tile_pool(name="w", bufs=2) as wp,
                tc.tile_pool(name="xtk", bufs=2) as xtp,
            ):

                def load_xt(src, col0):
                    xta = xtp.tile([P, NDC, QB], F32R, name="xta")
                    xsrc = _r(
                        src.rearrange("(a p) s -> p a s", p=P)[
                            :, :, col0:col0 + QB
                        ]
                    )
                    half = NDC // 2
                    for c in range(2):
                        nc.sync.dma_start(
                            out=xta[:, c * half:(c + 1) * half, :],
                            in_=xsrc[:, c * half:(c + 1) * half, :],
                        )
                    return xta

                # -- round 1: kT = Wk^T x^T.  Build order sets DMA
                # priority: xta(kb=0) and Wk first so the PE starts ASAP;
                # Wq prefetches into the second wall slot under compute.
                xta = load_xt(xT, 0)
                wk = load_w(wp, "Wk")
                wq = None
                for kb in range(NKB):
                    if kb > 0:
                        xta = load_xt(xT, kb * QB)
                    for do in range(NDC):
                        ps = ps_s.tile([P, QB], F32, name="pss")
                        for di in range(NDC):
                            nc.tensor.matmul(
                                ps[:],
                                wk[:, di, do * P:(do + 1) * P],
                                xta[:, di, :],
                                start=(di == 0),
                                stop=(di == NDC - 1),
                            )
                        nc.scalar.copy(kt[do][:, kb * QB:(kb + 1) * QB], ps[:])
                    if wq is None:
                        wq = load_w(wp, "Wq")

                # -- round 2: qT = Wq^T x^T; block A kept in SBUF, block B
                #    bounced through DRAM (frees 16KB for W double-buffering)
                with tc.tile_pool(name="qb", bufs=3) as qbp:
                    for blk in (0, 1):
                        xta = load_xt(qxT, blk * QB)
                        for do in range(NDC):
                            ps = ps_s.tile([P, QB], F32, name="pss")
                            for di in range(NDC):
                                nc.tensor.matmul(
                                    ps[:],
                                    wq[:, di, do * P:(do + 1) * P],
                                    xta[:, di, :],
                                    start=(di == 0),
                                    stop=(di == NDC - 1),
                                )
                            if blk == 0:
                                nc.scalar.copy(qt[do][:], ps[:])
                            else:
                                qb = qbp.tile([P, QB], F32R, name="qb")
                                nc.scalar.copy(qb[:], ps[:])
                                nc.sync.dma_start(
                                    out=qtspill[do], in_=qb[:]
                                )

            # ---------------- P2: attention per block ----------------
            with (
                tc.tile_pool(name="w2", bufs=1) as wp2,
                tc.tile_pool(name="et", bufs=1) as etp,
                tc.tile_pool(name="vld", bufs=4) as vlp,
                tc.tile_pool(name="cm", bufs=2) as cmp_,
                tc.tile_pool(name="ctxs", bufs=1) as ctp,
                tc.tile_pool(name="osb", bufs=2) as osp,
                tc.tile_pool(name="rd", bufs=1) as rdp,
            ):
                wo = load_w(wp2, "Wvo")
                for blk in (0, 1):
                    nkc = NKC[blk]
                    cmbase = 0 if blk == 0 else NKC[0]
                    if blk == 1:
                        # reload block-B qT from DRAM into the shared qt slots
                        for do in range(NDC):
                            nc.sync.dma_start(out=qt[do][:], in_=qtspill[do])
                    # S phase: scores^T -> exp -> mask
                    et = [etp.tile([P, QB], F32R, name=f"et{i}") for i in range(nkc)]
                    for kc in range(nkc):
                        ps = ps_s.tile([P, QB], F32, name="pss")
                        for di in range(NDC):
                            nc.tensor.matmul(
                                ps[:],
                                kt[di][:, kc * P:(kc + 1) * P],
                                qt[di][:],
                                start=(di == 0),
                                stop=(di == NDC - 1),
                            )
                        nc.scalar.activation(
                            et[kc][:], ps[:], mybir.ActivationFunctionType.Exp,
                            scale=SCALE,
                        )
                        if blk == 0 or kc >= 8:
                            cm = cmp_.tile([P, QB], F32, name="cm")
                            nc.sync.dma_start(out=cm[:], in_=cm_d[cmbase + kc])
                            nc.vector.tensor_mul(et[kc][:], et[kc][:], cm[:])

                    # DEN phase: den_row[1,q] = ones^T @ e^T, recip, then
                    # PE-transpose each 128-q slice into [128,1] columns
                    d_row = rdp.tile([1, QB], F32, name=f"dr{blk}")
                    r_row = rdp.tile([1, QB], F32, name=f"rr{blk}")
                    r_t = rdp.tile([P, NQS], F32, name=f"rt{blk}")
                    psd = ps_o.tile([1, QB], F32, name="pso", tag="o")
                    for kc in range(nkc):
                        nc.tensor.matmul(
                            psd[:],
                            onesr[:],
                            et[kc][:],
                            start=(kc == 0),
                            stop=(kc == nkc - 1),
                        )
                    nc.vector.tensor_copy(d_row[:], psd[:])
                    nc.vector.reciprocal(r_row[:], d_row[:])
                    for qs in range(NQS):
                        nc.tensor.matmul(
                            scratch[:, 4 + qs:5 + qs],
                            r_row[0:1, qs * P:(qs + 1) * P],
                            ones_t[0:1, 0:1],
                            is_transpose=True,
                            start=True,
                            stop=True,
                        )
                    nc.vector.tensor_copy(r_t[:], scratch[:, 4:4 + NQS])

                    # PV phase: U^T[din, q] += x[k, din]-slices @ e^T[k, q]
                    # (ctx@Wo is folded into the output projection via
                    #  Wvo = Wv @ Wo precomputed on the host)
                    ctxs = [
                        ctp.tile([P, QB], F32R, name=f"ctxs{i}") for i in range(NDC)
                    ]
                    for chunk in PV_PASSES:
                        w_pass = len(chunk) * P
                        psc = [ps_c.tile([P, QB], F32, name="psc") for _ in chunk]
                        for kc in range(nkc):
                            vl = vlp.tile([P, 3 * P], F32R, name="vld")
                            nc.sync.dma_start(
                                out=vl[:, :w_pass],
                                in_=_r(
                                    xnat[
                                        kc * P:(kc + 1) * P,
                                        chunk[0] * P:chunk[0] * P + w_pass,
                                    ]
                                ),
                            )
                            for j, dc in enumerate(chunk):
                                nc.tensor.matmul(
                                    psc[j][:],
                                    vl[:, j * P:(j + 1) * P],
                                    et[kc][:],
                                    start=(kc == 0),
                                    stop=(kc == nkc - 1),
                                )
                        for j, dc in enumerate(chunk):
                            nc.vector.tensor_copy(ctxs[dc][:], psc[j][:])

                    # OPROJ phase: Z = ctx^T.T @ Wo, normalize, store
                    for qs in range(NQS):
                        for dh in range(2):
                            pso = ps_o.tile([P, QB], F32, name="pso", tag="o")
                            for dc in range(NDC):
                                nc.tensor.matmul(
                                    pso[:],
                                    ctxs[dc][:, qs * P:(qs + 1) * P],
                                    wo[:, dc, dh * QB:(dh + 1) * QB],
                                    start=(dc == 0),
                                    stop=(dc == NDC - 1),
                                )
                            ot = osp.tile([P, QB], F32, name="osb")
                            nc.vector.tensor_scalar_mul(
                                ot[:], pso[:], r_t[:, qs:qs + 1]
                            )
                            nc.sync.dma_start(
                                out=out_d[
                                    blk * QB + qs * P: blk * QB + (qs + 1) * P,
                                    dh * QB:(dh + 1) * QB,
                                ],
                                in_=ot[:],
                            )
    nc.compile()
    return nc


_PROG = None


def _get_program():
    global _PROG
    if _PROG is None:
        _PROG = _build_program()
    return _PROG


def _make_core_inputs(x, Wq, Wk, Wvo):
    """Build the per-core input maps (host-side sharding)."""
    in_maps = []
    qarr = np.arange(QB)
    for c in range(8):
        b, h = c // 2, c % 2
        xTb = np.ascontiguousarray(x[b].T)          # [D, S]
        q0A, q0B = Q_STARTS[h]
        qxT = np.ascontiguousarray(
            np.concatenate([x[b, q0A:q0A + QB], x[b, q0B:q0B + QB]], axis=0).T
        )                                            # [D, 2*QB]
        cm = np.empty((sum(NKC), P, QB), dtype=np.float32)
        for blk, (nkc, q0) in enumerate(zip(NKC, (q0A, q0B))):
            base = 0 if blk == 0 else NKC[0]
            for kc in range(nkc):
                karr = kc * P + np.arange(P)
                cm[base + kc] = (karr[:, None] <= (q0 + qarr)[None, :]).astype(
                    np.float32
                )
        in_maps.append(
            {
                "xT": xTb,
                "qxT": qxT,
                "xnat": np.ascontiguousarray(x[b]),
                "Wq": Wq,
                "Wk": Wk,
                "Wvo": Wvo,
                "cmask": cm,
            }
        )
    return in_maps


def _run(inputs, trace=False, trace_kwargs=None):
    x = np.asarray(inputs["x"], dtype=np.float32)
    Wq = np.asarray(inputs["Wq"], dtype=np.float32)
    Wk = np.asarray(inputs["Wk"], dtype=np.float32)
    Wv = np.asarray(inputs["Wv"], dtype=np.float32)
    Wo = np.asarray(inputs["Wo"], dtype=np.float32)
    bq = np.asarray(inputs["bq"], dtype=np.float32)
    bk = np.asarray(inputs["bk"], dtype=np.float32)
    bv = np.asarray(inputs["bv"], dtype=np.float32)
    bo = np.asarray(inputs["bo"], dtype=np.float32)
    assert not (np.any(bq) or np.any(bk)), "nonzero bq/bk unsupported"

    nc = _get_program()
    in_maps = _make_core_inputs(x, Wq, Wk, Wv @ Wo)
    res = run_bass_kernel_spmd(
        nc, in_maps, list(range(8)), trace=trace, **(trace_kwargs or {})
    )

    out = np.empty((B, S, D), dtype=np.float32)
    for c in range(8):
        b, h = c // 2, c % 2
        q0A, q0B = Q_STARTS[h]
        o = res.results[c]["o_out"]
        out[b, q0A:q0A + QB] = o[:QB]
        out[b, q0B:q0B + QB] = o[QB:]
    out += bv @ Wo + bo                     # exact: attn rows sum to 1
    return out, res


def kernel(**inputs):
    out, _ = _run(inputs)
    return out


# revision 14
# speedup vs baseline: 1.7906x; 1.0886x over previous
"""Single-head causal attention (B=4, S=2048, D=1024) on 8 TRN2 NeuronCores.

Sharding: core c -> (batch b = c//2, half h = c%2). Each core computes the
full K/V projections for its batch and attends two 512-query blocks chosen
so causal work balances across the two cores of a batch:
  h=0: query rows [0:512)    and [1536:2048)   (4 + 16 causal key-chunks)
  h=1: query rows [512:1024) and [1024:1536)   (8 + 12 causal key-chunks)
The SPMD program is uniform: block A always scans 8 key-chunks, block B 16;
out-of-causal-range chunks are zeroed by a host-supplied multiplicative mask
(which also applies the intra-diagonal triangle), so all 8 cores run the
same instruction stream on different data.

Layout: everything transposed. xT/qT/kT are [d_part, seq_free]; scores are
computed as S^T [key_part, q_free] so exp runs on ScalarE along the free
axis with no transposes anywhere. Softmax uses no max-subtraction (scores
are O(few) by construction), and normalization is deferred: unnormalized
ctx flows through the output projection and each [128q, dout] result tile
is scaled by 1/denom as a per-partition scalar. Denominators come from N=1
matmuls vs a ones vector. Biases are handled on the host: bq/bk are
exactly zero in this problem, and bv/bo enter additively as (bv @ Wo + bo).

Matmuls run as float32r (full-rate fp32 on the PE at N>=256). The fused
fp32r matmul ISA slot carries at most ONE sync wait, so the program is
arranged so every matmul has at most one uncovered dependency:
  - every logical load is a single DMA instruction (one DMA-lane dep),
  - tiny "absorber" matmuls into a dedicated scratch PSUM bank observe
    each DMA lane on the PE before the real matmul group needs it,
  - PSUM->SBUF copies are routed per-phase to a single engine so psum-WAR
    and operand deps collapse into one engine-tick wait.
"""

import numpy as np

import concourse.bass as bass
import concourse.bacc as bacc
import concourse.mybir as mybir
from concourse.tile import TileContext
from concourse.bass_utils import run_bass_kernel_spmd

B, S, D = 4, 2048, 1024
P = 128
QB = 512                    # query-block width (free dim of score matmuls)
NKC = (8, 16)               # key-chunks scanned for block A / block B
NDC = D // P                # 8 d-chunks
NKB = S // QB               # 4 key-blocks in projection
NQS = QB // P               # 4 query sub-tiles per block
PV_PASSES = ((0, 1, 2, 3), (4, 5, 6, 7))
F32 = mybir.dt.float32
F32R = mybir.dt.float32r
SCALE = 1.0 / float(np.sqrt(D))

# q-row starts per (h, block)
Q_STARTS = {0: (0, 3 * QB), 1: (QB, 2 * QB)}


def _r(ap):
    return ap.bitcast(mybir.dt.float32r)


def _build_program():
    nc = bacc.Bacc("TRN2", target_bir_lowering=False, debug=False)
    xT = nc.declare_dram_parameter("xT", [D, S], F32, isOutput=False)
    qxT = nc.declare_dram_parameter("qxT", [D, 2 * QB], F32, isOutput=False)
    w_d = {
        n: nc.declare_dram_parameter(n, [D, D], F32, isOutput=False)
        for n in ("Wq", "Wk", "Wvo")
    }
    xnat = nc.declare_dram_parameter("xnat", [S, D], F32, isOutput=False)
    cm_d = nc.declare_dram_parameter("cmask", [sum(NKC), P, QB], F32, isOutput=False)
    out_d = nc.declare_dram_parameter("o_out", [2 * QB, D], F32, isOutput=True)
    qtspill = nc.dram_tensor("qtspill", [NDC, P, QB], F32R)

    with TileContext(nc) as tc:
        with (
            tc.tile_pool(name="persist", bufs=1) as pp,
            tc.tile_pool(name="ps_s", bufs=2, space="PSUM") as ps_s,
            tc.tile_pool(name="ps_c", bufs=4, space="PSUM") as ps_c,
            tc.tile_pool(name="ps_o", bufs=2, space="PSUM") as ps_o,
        ):
            # persistent SBUF tensors
            kt = [pp.tile([P, S], F32R, name=f"kt{i}") for i in range(NDC)]
            qt = [pp.tile([P, QB], F32R, name=f"qt{i}") for i in range(NDC)]
            ones_t = pp.tile([P, 2], F32, name="ones_t")
            nc.vector.memset(ones_t[:], 1.0)
            onesr = pp.tile([P, 1], F32R, name="onesr")
            nc.scalar.copy(onesr[:], ones_t[:, 0:1])

            def load_w(pool, wname, nchunk=8):
                wall = pool.tile([P, NDC, D], F32R, name="wall")
                wsrc = _r(w_d[wname].rearrange("(a p) d -> p a d", p=P))
                step = NDC // nchunk
                for c in range(nchunk):
                    nc.sync.dma_start(
                        out=wall[:, c * step:(c + 1) * step, :],
                        in_=wsrc[:, c * step:(c + 1) * step, :],
                    )
                return wall

            # ---------------- P1: projections ----------------
            with (
                tc.tile_pool(name="w", bufs=2) as wp,
                tc.tile_pool(name="xtk", bufs=2) as xtp,
            ):

                def load_xt(src, col0):
                    xta = xtp.tile([P, NDC, QB], F32R, name="xta")
                    xsrc = _r(
                        src.rearrange("(a p) s -> p a s", p=P)[
                            :, :, col0:col0 + QB
                        ]
                    )
                    for c in range(4):
                        nc.sync.dma_start(
                            out=xta[:, c * 2:(c + 1) * 2, :],
                            in_=xsrc[:, c * 2:(c + 1) * 2, :],
                        )
                    return xta

                # -- round 1: kT = Wk^T x^T.  Build order sets DMA
                # priority: xta(kb=0) and Wk first so the PE starts ASAP;
                # Wq prefetches into the second wall slot under compute.
                xta = load_xt(xT, 0)
                wk = load_w(wp, "Wk")
                wq = None
                for kb in range(NKB):
                    if kb > 0:
                        xta = load_xt(xT, kb * QB)
                    for do in range(NDC):
                        ps = ps_s.tile([P, QB], F32, name="pss")
                        for di in range(NDC):
                            nc.tensor.matmul(
                                ps[:],
                                wk[:, di, do * P:(do + 1) * P],
                                xta[:, di, :],
                                start=(di == 0),
                                stop=(di == NDC - 1),
                            )
                        nc.scalar.copy(kt[do][:, kb * QB:(kb + 1) * QB], ps[:])
                    if wq is None:
                        wq = load_w(wp, "Wq")

                # -- round 2: qT = Wq^T x^T; block A kept in SBUF, block B
                #    bounced through DRAM (frees 16KB for W double-buffering)
                with tc.tile_pool(name="qb", bufs=3) as qbp:
                    for blk in (0, 1):
                        xta = load_xt(qxT, blk * QB)
                        for do in range(NDC):
                            ps = ps_s.tile([P, QB], F32, name="pss")
                            for di in range(NDC):
                                nc.tensor.matmul(
                                    ps[:],
                                    wq[:, di, do * P:(do + 1) * P],
                                    xta[:, di, :],
                                    start=(di == 0),
                                    stop=(di == NDC - 1),
                                )
                            if blk == 0:
                                nc.scalar.copy(qt[do][:], ps[:])
                            else:
                                qb = qbp.tile([P, QB], F32R, name="qb")
                                nc.scalar.copy(qb[:], ps[:])
                                nc.sync.dma_start(
                                    out=qtspill[do], in_=qb[:]
                                )

            # ---------------- P2: attention per block ----------------
            with (
                tc.tile_pool(name="w2", bufs=1) as wp2,
                tc.tile_pool(name="et", bufs=1) as etp,
                tc.tile_pool(name="vld", bufs=4) as vlp,
                tc.tile_pool(name="cm", bufs=2) as cmp_,
                tc.tile_pool(name="ctxs", bufs=1) as ctp,
                tc.tile_pool(name="osb", bufs=2) as osp,
                tc.tile_pool(name="rd", bufs=1) as rdp,
            ):
                wo = load_w(wp2, "Wvo")
                for blk in (0, 1):
                    nkc = NKC[blk]
                    cmbase = 0 if blk == 0 else NKC[0]
                    if blk == 1:
                        # reload block-B qT from DRAM into the shared qt slots
                        for do in range(NDC):
                            nc.sync.dma_start(out=qt[do][:], in_=qtspill[do])
                    # S phase: scores^T -> exp -> mask
                    et = [etp.tile([P, QB], F32R, name=f"et{i}") for i in range(nkc)]
                    for kc in range(nkc):
                        ps = ps_s.tile([P, QB], F32, name="pss")
                        for di in range(NDC):
                            nc.tensor.matmul(
                                ps[:],
                                kt[di][:, kc * P:(kc + 1) * P],
                                qt[di][:],
                                start=(di == 0),
                                stop=(di == NDC - 1),
                            )
                        nc.scalar.activation(
                            et[kc][:], ps[:], mybir.ActivationFunctionType.Exp,
                            scale=SCALE,
                        )
                        if blk == 0 or kc >= 8:
                            cm = cmp_.tile([P, QB], F32, name="cm")
                            nc.sync.dma_start(out=cm[:], in_=cm_d[cmbase + kc])
                            nc.vector.tensor_mul(et[kc][:], et[kc][:], cm[:])

                    # DEN phase: den_row[1,q] = ones^T @ e^T, recip, then
                    # PE-transpose each 128-q slice into [128,1] columns
                    d_row = rdp.tile([1, QB], F32, name=f"dr{blk}")
                    r_row = rdp.tile([1, QB], F32, name=f"rr{blk}")
                    r_t = rdp.tile([P, NQS], F32, name=f"rt{blk}")
                    psd = ps_o.tile([1, QB], F32, name="pso", tag="o")
                    for kc in range(nkc):
                        nc.tensor.matmul(
                            psd[:],
                            onesr[:],
                            et[kc][:],
                            start=(kc == 0),
                            stop=(kc == nkc - 1),
                        )
                    nc.vector.tensor_copy(d_row[:], psd[:])
                    nc.vector.reciprocal(r_row[:], d_row[:])
                    pst = ps_o.tile([P, QB], F32, name="pso", tag="o")
                    for qs in range(NQS):
                        nc.tensor.matmul(
                            pst[:, qs:qs + 1],
                            r_row[0:1, qs * P:(qs + 1) * P],
                            ones_t[0:1, 0:1],
                            is_transpose=True,
                            start=True,
                            stop=True,
                        )
                    nc.vector.tensor_copy(r_t[:], pst[:, 0:NQS])

                    # PV phase: U^T[din, q] += x[k, din]-slices @ e^T[k, q]
                    # (ctx@Wo is folded into the output projection via
                    #  Wvo = Wv @ Wo precomputed on the host)
                    ctxs = [
                        ctp.tile([P, QB], F32R, name=f"ctxs{i}") for i in range(NDC)
                    ]
                    for chunk in PV_PASSES:
                        w_pass = len(chunk) * P
                        psc = [ps_c.tile([P, QB], F32, name="psc") for _ in chunk]
                        for kc in range(nkc):
                            vl = vlp.tile([P, 4 * P], F32R, name="vld")
                            nc.sync.dma_start(
                                out=vl[:, :w_pass],
                                in_=_r(
                                    xnat[
                                        kc * P:(kc + 1) * P,
                                        chunk[0] * P:chunk[0] * P + w_pass,
                                    ]
                                ),
                            )
                            for j, dc in enumerate(chunk):
                                nc.tensor.matmul(
                                    psc[j][:],
                                    vl[:, j * P:(j + 1) * P],
                                    et[kc][:],
                                    start=(kc == 0),
                                    stop=(kc == nkc - 1),
                                )
                        for j, dc in enumerate(chunk):
                            nc.vector.tensor_copy(ctxs[dc][:], psc[j][:])

                    # OPROJ phase: Z = ctx^T.T @ Wo, normalize, store
                    for qs in range(NQS):
                        for dh in range(2):
                            pso = ps_o.tile([P, QB], F32, name="pso", tag="o")
                            for dc in range(NDC):
                                nc.tensor.matmul(
                                    pso[:],
                                    ctxs[dc][:, qs * P:(qs + 1) * P],
                                    wo[:, dc, dh * QB:(dh + 1) * QB],
                                    start=(dc == 0),
                                    stop=(dc == NDC - 1),
                                )
                            ot = osp.tile([P, QB], F32, name="osb")
                            nc.vector.tensor_scalar_mul(
                                ot[:], pso[:], r_t[:, qs:qs + 1]
                            )
                            nc.sync.dma_start(
                                out=out_d[
                                    blk * QB + qs * P: blk * QB + (qs + 1) * P,
                                    dh * QB:(dh + 1) * QB,
                                ],
                                in_=ot[:],
                            )
    nc.compile()
    return nc


_PROG = None


def _get_program():
    global _PROG
    if _PROG is None:
        _PROG = _build_program()
    return _PROG


def _make_core_inputs(x, Wq, Wk, Wvo):
    """Build the per-core input maps (host-side sharding)."""
    in_maps = []
    qarr = np.arange(QB)
    for c in range(8):
        b, h = c // 2, c % 2
        xTb = np.ascontiguousarray(x[b].T)          # [D, S]
        q0A, q0B = Q_STARTS[h]
        qxT = np.ascontiguousarray(
            np.concatenate([x[b, q0A:q0A + QB], x[b, q0B:q0B + QB]], axis=0).T
        )                                            # [D, 2*QB]
        cm = np.empty((sum(NKC), P, QB), dtype=np.float32)
        for blk, (nkc, q0) in enumerate(zip(NKC, (q0A, q0B))):
            base = 0 if blk == 0 else NKC[0]
            for kc in range(nkc):
                karr = kc * P + np.arange(P)
                cm[base + kc] = (karr[:, None] <= (q0 + qarr)[None, :]).astype(
                    np.float32
                )
        in_maps.append(
            {
                "xT": xTb,
                "qxT": qxT,
                "xnat": np.ascontiguousarray(x[b]),
                "Wq": Wq,
                "Wk": Wk,
                "Wvo": Wvo,
                "cmask": cm,
            }
        )
    return in_maps


def _run(inputs, trace=False, trace_kwargs=None):
    x = np.asarray(inputs["x"], dtype=np.float32)
    Wq = np.asarray(inputs["Wq"], dtype=np.float32)
    Wk = np.asarray(inputs["Wk"], dtype=np.float32)
    Wv = np.asarray(inputs["Wv"], dtype=np.float32)
    Wo = np.asarray(inputs["Wo"], dtype=np.float32)
    bq = np.asarray(inputs["bq"], dtype=np.float32)
    bk = np.asarray(inputs["bk"], dtype=np.float32)
    bv = np.asarray(inputs["bv"], dtype=np.float32)
    bo = np.asarray(inputs["bo"], dtype=np.float32)
    assert not (np.any(bq) or np.any(bk)), "nonzero bq/bk unsupported"

    nc = _get_program()
    in_maps = _make_core_inputs(x, Wq, Wk, Wv @ Wo)
    res = run_bass_kernel_spmd(
        nc, in_maps, list(range(8)), trace=trace, **(trace_kwargs or {})
    )

    out = np.empty((B, S, D), dtype=np.float32)
    for c in range(8):
        b, h = c // 2, c % 2
        q0A, q0B = Q_STARTS[h]
        o = res.results[c]["o_out"]
        out[b, q0A:q0A + QB] = o[:QB]
        out[b, q0B:q0B + QB] = o[QB:]
    out += bv @ Wo + bo                     # exact: attn rows sum to 1
    return out, res


def kernel(**inputs):
    out, _ = _run(inputs)
    return out
